# revision 1
# baseline (speedup 1.0000x reference)
"""FBPINN (windowed mixture of per-subdomain MLPs) Trainium2 kernel.

Strategy: the cosine partition-of-unity window has compact support — a
point contributes to a subdomain's MLP only if it lies strictly inside
that subdomain's box.  With the 8x4 overlapped tiling each point lands in
~2.5 of the 32 boxes, so dense evaluation wastes >90% of the FLOPs.

  host:   route points -> per-expert gathered (padded) point lists
  device: 8 cores x 4 experts each; experts packed in pairs into the
          128-partition systolic array (block-diagonal weights); tiny
          MLP in feature-major layout, tanh+bias fused on the ACT engine
          reading PSUM directly.
  host:   scatter-add  w*(o+bo)  and  w  per point, final normalize.

Matmul operands use float32r (single-pass PE streaming, 4x the fp32
rate, ~11-bit effective multiplier).  Layer 0 stays accurate because x
and the folded W0 are hi/lo bf16-split on the host (the hi parts pass
through the reduced multiplier exactly, and the four cross terms
reconstruct the full product); K grows 4->16 which is free since PE
cost scales with moving columns, not K.  Experts are pair-slot balanced
by point count so padding is minimal; DMA traffic is packed to minimize
per-dma_start queue-issue overhead; the tanh spline table is preloaded
and the PE clock gate warmed while the first inputs stream in.
"""

import numpy as np

import concourse.bacc as bacc
import concourse.mybir as mybir
import concourse.tile as tile
from concourse.bass_utils import run_bass_kernel_spmd

# problem constants (hardcoded per contract)
N_PTS = 32768
S = 32
XDIM = 2
WIDTH = 64
TRANS = 0.1
TOL = 1e-8
N_CORES = 8
E_PER_CORE = S // N_CORES      # 4 experts per core
PAIRS = E_PER_CORE // 2        # 2 block-diag pairs (slots) per core

MM_CH = 512                    # fp32 moving-operand max per matmul
ACT_CH = 1536                  # ACT reads 3 psum banks per instruction
K0 = 16                        # L0 contraction rows after hi/lo splitting

# packed per-pair weight layout (128 partitions x WCOLS):
#   [0:128)   W1 block-diag   [128:256) W2 block-diag
#   [256:384) W0 (rows 0:K0)  [384:386) Wo columns
WCOLS = 386

_compiled_cache: dict[tuple, object] = {}


def _build_nc(slot_pads: tuple[int, ...]):
    fp32 = mybir.dt.float32
    fp32r = mybir.dt.float32r
    nc = bacc.Bacc("TRN2", target_bir_lowering=False, debug=False,
                   num_devices=N_CORES)

    wp_d = nc.dram_tensor("wpack", [PAIRS, 128, WCOLS], fp32r,
                          kind="ExternalInput")
    bias_d = nc.dram_tensor("bias", [PAIRS, 128, 3], fp32,
                            kind="ExternalInput")
    xt_d = [nc.dram_tensor(f"xt{p}", [K0, slot_pads[p]], fp32r,
                           kind="ExternalInput") for p in range(PAIRS)]
    oo_d = [nc.dram_tensor(f"oo{p}", [2, slot_pads[p]], fp32,
                           kind="ExternalOutput") for p in range(PAIRS)]
    # raw h2 of the overall-last 256 columns: their output layer runs on
    # the host, so the post-stream tail skips the PE->DVE->DMA chain
    h2t_d = nc.dram_tensor("h2t", [128, 256], fp32r, kind="ExternalOutput")

    tanh = mybir.ActivationFunctionType.Tanh

    with tile.TileContext(nc) as tc:
        with (
            tc.tile_pool(name="wpool", bufs=2) as wpool,
            tc.tile_pool(name="hpool", bufs=2) as hpool,
            tc.tile_pool(name="ps", bufs=2, space="PSUM") as ps,
            tc.tile_pool(name="pso", bufs=2, space="PSUM") as psop,
        ):
            # PE warm-up during the input-DMA window: the HAM clock gate
            # starts at half rate and releases after ~3us of activity.
            bf16 = mybir.dt.bfloat16
            wm = wpool.tile([128, MM_CH], bf16, tag="wm")
            nc.vector.memset(wm[:], 0.0)
            # preload the tanh spline table while DMAs stream
            nc.scalar.activation(wm[0:1, 0:1], wm[0:1, 0:1], tanh)
            for _ in range(3):
                pwm = ps.tile([128, ACT_CH], fp32, tag="ps")
                nc.tensor.matmul(pwm[:, 0:MM_CH], wm[:, 0:128], wm[:],
                                 start=True, stop=True)

            for p in range(PAIRS):
                pad = slot_pads[p]
                nch = pad // MM_CH
                wp = wpool.tile([128, WCOLS], fp32r, tag="wp")
                bt = wpool.tile([128, 3], fp32, tag="bt")
                xt = hpool.tile([K0, pad], fp32r, tag="xt")
                # latency-critical transfers on the HWDGE ring in need-order;
                # the big W1/W2 block on SWDGE (Pool) in parallel
                nc.sync.dma_start(xt[:, 0:ACT_CH], xt_d[p][:, 0:ACT_CH])
                nc.sync.dma_start(wp[0:K0, 256:384], wp_d[p][0:K0, 256:384])
                nc.sync.dma_start(bt[:], bias_d[p])
                nc.gpsimd.dma_start(wp[:, 0:256], wp_d[p][:, 0:256])
                nc.gpsimd.dma_start(wp[:, 384:386], wp_d[p][:, 384:386])
                for a0 in range(ACT_CH, pad, ACT_CH):
                    alen = min(ACT_CH, pad - a0)
                    nc.sync.dma_start(xt[:, a0:a0 + alen],
                                      xt_d[p][:, a0:a0 + alen])
                w1 = wp[:, 0:128]
                w2 = wp[:, 128:256]
                w0 = wp[0:K0, 256:384]
                wo = wp[:, 384:386]
                b0 = bt[:, 0:1]
                b1 = bt[:, 1:2]
                b2 = bt[:, 2:3]

                h0 = hpool.tile([128, pad], fp32r, tag="h0")
                h1 = hpool.tile([128, pad], fp32r, tag="h1")
                h2 = hpool.tile([128, pad], fp32r, tag="h2")

                # ACT-chunk grid (ragged tail allowed); wide chunks
                # amortize the 222-cycle SBUF-access overhead per ACT
                acts = []
                a0 = 0
                while a0 < pad:
                    acts.append((a0, min(ACT_CH, pad - a0)))
                    a0 += ACT_CH

                # layer-outer emission: PE (in-order FIFO) streams a whole
                # layer's chunks while ACT drains the previous layer, so
                # the two engines pipeline instead of ping-ponging.
                # L2 of the overall-last pair ends with a small chunk so
                # the closing h2-tail DMA fires as early as possible
                acts_l2 = acts
                last = p == PAIRS - 1
                if last:
                    acts_l2 = []
                    a0 = 0
                    while a0 < pad - 256:
                        acts_l2.append((a0, min(ACT_CH, pad - 256 - a0)))
                        a0 += ACT_CH
                    acts_l2.append((pad - 256, 256))
                for lw, lb, src, dst in ((w0, b0, xt, h0), (w1, b1, h0, h1),
                                         (w2, b2, h1, h2)):
                    for a0, alen in (acts_l2 if dst is h2 else acts):
                        pst = ps.tile([128, ACT_CH], fp32, tag="ps")
                        for m in range(0, alen, MM_CH):
                            ml = min(MM_CH, alen - m)
                            nc.tensor.matmul(
                                pst[:, m:m + ml], lw,
                                src[:, a0 + m:a0 + m + ml],
                                start=True, stop=True)
                        nc.scalar.activation(dst[:, a0:a0 + alen],
                                             pst[:, 0:alen], tanh, bias=lb)

                o_sb = hpool.tile([2, pad], fp32, tag="o_sb")
                o_end = pad - 256 if last else pad
                ci = 0
                for g0 in range(0, o_end, ACT_CH):
                    glen = min(ACT_CH, o_end - g0)
                    pst_tail = None
                    for m in range(0, glen, MM_CH):
                        o = g0 + m
                        ml = min(MM_CH, glen - m)
                        # the last pair's Lout runs post-stream: avoid the
                        # pso WAR rotation by parking late chunks in a ps
                        # tile, and split copies across DVE + idle ACT
                        if last and ci >= 2:
                            if pst_tail is None:
                                pst_tail = ps.tile([128, ACT_CH], fp32,
                                                   tag="ps")
                            pso = pst_tail[0:2, m:m + MM_CH]
                        else:
                            pso_t = psop.tile([2, MM_CH], fp32, tag="pso")
                            pso = pso_t[:]
                        nc.tensor.matmul(pso[:, 0:ml], wo, h2[:, o:o + ml],
                                         start=True, stop=True)
                        if last and ci % 2 == 1:
                            nc.scalar.activation(
                                o_sb[:, o:o + ml], pso[:, 0:ml],
                                mybir.ActivationFunctionType.Copy)
                        else:
                            nc.vector.tensor_copy(o_sb[:, o:o + ml],
                                                  pso[:, 0:ml])
                        ci += 1
                    if not last:
                        nc.sync.dma_start(oo_d[p][:, g0:g0 + glen],
                                          o_sb[:, g0:g0 + glen])
                if last:
                    # single egress after all copies: one ring slot, and
                    # the sync ring is idle again by then
                    nc.sync.dma_start(oo_d[p][:, 0:o_end], o_sb[:, 0:o_end])
                if last:
                    nc.sync.dma_start(h2t_d[:], h2[:, pad - 256:pad])
    nc.compile()
    return nc


def _get_nc(slot_pads):
    key = tuple(slot_pads)
    nc = _compiled_cache.get(key)
    if nc is None:
        nc = _build_nc(key)
        _compiled_cache[key] = nc
    return nc


def _assign_experts(counts):
    """Pair experts and assign to (core, slot) balancing point counts.

    Returns assign[core][slot] = (expert_a, expert_b) and slot_pads.
    Sort experts by count desc; adjacent pairing minimizes within-pair
    padding; the 8 largest pairs go to slot 0, the rest to slot 1, so
    each slot's cross-core pad (max over cores) stays tight.
    """
    order = sorted(range(S), key=lambda s: (-counts[s], s))
    pairs = [(order[2 * i], order[2 * i + 1]) for i in range(S // 2)]
    pairs.sort(key=lambda ab: -max(counts[ab[0]], counts[ab[1]]))
    assign = [[None] * PAIRS for _ in range(N_CORES)]
    slot_pads = []
    for p in range(PAIRS):
        chunk = pairs[p * N_CORES:(p + 1) * N_CORES]
        mx = max(max(counts[a], counts[b]) for a, b in chunk)
        slot_pads.append(int(max(MM_CH, -(-int(mx) // 128) * 128)))
        for c in range(N_CORES):
            assign[c][p] = chunk[c]
    return assign, tuple(slot_pads)


def _kernel_numpy(x, xmins, xmaxs, W0, b0, W1, b1, W2, b2, Wo, bo):
    """Dense reference fallback (correct for any shapes, host-only)."""
    x = np.asarray(x, np.float64)
    xmins = np.asarray(xmins, np.float64)
    xmaxs = np.asarray(xmaxs, np.float64)
    xe = x[:, None, :]
    tu = np.clip((xe - xmins) / TRANS, 0.0, 1.0)
    td = np.clip((xmaxs - xe) / TRANS, 0.0, 1.0)
    w = (0.25 * (1.0 - np.cos(np.pi * tu))
         * (1.0 - np.cos(np.pi * td))).prod(-1)
    w = w / (w.sum(1, keepdims=True) + TOL)
    center = 0.5 * (xmins + xmaxs)
    scale = np.maximum(0.5 * (xmaxs - xmins), 1e-9)
    xn = (xe - center) / scale
    h = np.tanh(np.einsum("nsd,shd->nsh", xn, np.asarray(W0, np.float64))
                + np.asarray(b0, np.float64))
    h = np.tanh(np.einsum("nsh,skh->nsk", h, np.asarray(W1, np.float64))
                + np.asarray(b1, np.float64))
    h = np.tanh(np.einsum("nsh,skh->nsk", h, np.asarray(W2, np.float64))
                + np.asarray(b2, np.float64))
    out = (np.einsum("nsh,soh->nso", h, np.asarray(Wo, np.float64))
           + np.asarray(bo, np.float64))
    y = (out * w[:, :, None]).sum(1)
    return y.astype(np.float32)


def kernel(x, xmins, xmaxs, W0, b0, W1, b1, W2, b2, Wo, bo):
    import ml_dtypes

    x = np.asarray(x)
    n_pts = x.shape[0]
    args = (x, xmins, xmaxs, W0, b0, W1, b1, W2, b2, Wo, bo)
    if (x.shape != (N_PTS, XDIM) or np.asarray(xmins).shape != (S, XDIM)
            or np.asarray(W0).shape != (S, WIDTH, XDIM)):
        return _kernel_numpy(*args)

    xmins64 = np.asarray(xmins, np.float64)
    xmaxs64 = np.asarray(xmaxs, np.float64)
    x64 = np.asarray(x, np.float64)

    # ---- host routing: strict-interior membership == window support ----
    inside = ((x[:, None, :] > xmins[None, :, :])
              & (x[:, None, :] < xmaxs[None, :, :])).all(-1)      # (N, S)
    idx = [np.nonzero(inside[:, s])[0] for s in range(S)]
    counts = np.array([len(i) for i in idx])
    assign, slot_pads = _assign_experts(counts)
    if max(slot_pads) > 6144:
        # degenerate clustering would overflow SBUF tiles; stay correct
        return _kernel_numpy(*args)

    # ---- fold input normalization into layer-0 weights (float64) ----
    center = 0.5 * (xmins64 + xmaxs64)                            # (S, 2)
    scale = np.maximum(0.5 * (xmaxs64 - xmins64), 1e-9)
    W0f = np.asarray(W0, np.float64) / scale[:, None, :]          # (S, 64, 2)
    b0f = np.asarray(b0, np.float64) - (W0f * center[:, None, :]).sum(-1)

    def _split(v):
        hi = np.asarray(v, np.float32).astype(ml_dtypes.bfloat16) \
            .astype(np.float32)
        return hi, (np.asarray(v, np.float32) - hi)

    W1 = np.asarray(W1)
    W2 = np.asarray(W2)
    Wo = np.asarray(Wo)
    b1 = np.asarray(b1)
    b2 = np.asarray(b2)
    in_maps = []
    for core in range(N_CORES):
        m = {"wpack": np.zeros((PAIRS, 128, WCOLS), np.float32),
             "bias": np.zeros((PAIRS, 128, 3), np.float32)}
        wpk = m["wpack"]
        bk = m["bias"]
        for p in range(PAIRS):
            pad = slot_pads[p]
            xt = np.zeros((K0, pad), np.float32)
            for j, s in enumerate(assign[core][p]):
                lo, hi = 64 * j, 64 * (j + 1)
                pts = x[idx[s]]                                   # (P_s, 2)
                x_hi, x_lo = _split(pts.T)                        # (2, P_s)
                w_hi, w_lo = _split(W0f[s].T)                     # (2, 64)
                r0 = 8 * j
                n = pts.shape[0]
                # rows: [x_hi|W_hi, x_lo|W_hi, x_hi|W_lo, x_lo|W_lo]
                xt[r0 + 0:r0 + 2, :n] = x_hi
                xt[r0 + 2:r0 + 4, :n] = x_lo
                xt[r0 + 4:r0 + 6, :n] = x_hi
                xt[r0 + 6:r0 + 8, :n] = x_lo
                wpk[p, lo:hi, 0 + lo:0 + hi] = W1[s].T
                wpk[p, lo:hi, 128 + lo:128 + hi] = W2[s].T
                wpk[p, r0 + 0:r0 + 2, 256 + lo:256 + hi] = w_hi
                wpk[p, r0 + 2:r0 + 4, 256 + lo:256 + hi] = w_hi
                wpk[p, r0 + 4:r0 + 6, 256 + lo:256 + hi] = w_lo
                wpk[p, r0 + 6:r0 + 8, 256 + lo:256 + hi] = w_lo
                wpk[p, lo:hi, 384 + j] = Wo[s, 0, :]
                bk[p, lo:hi, 0] = b0f[s]
                bk[p, lo:hi, 1] = b1[s]
                bk[p, lo:hi, 2] = b2[s]
            m[f"xt{p}"] = xt
        in_maps.append(m)

    # ---- run on 8 cores ----
    global _last_in_maps
    _last_in_maps = in_maps
    try:
        nc = _get_nc(slot_pads)
        res = run_bass_kernel_spmd(nc, in_maps,
                                   core_ids=list(range(N_CORES)),
                                   trace=False)
    except Exception:
        import os
        if os.environ.get("BASS_KERNEL_NO_FALLBACK"):
            raise
        return _kernel_numpy(*args)

    # ---- window values + host scatter-add + normalize ----
    def window_vals(pts64, s):
        tu = np.clip((pts64 - xmins64[s]) / TRANS, 0.0, 1.0)
        td = np.clip((xmaxs64[s] - pts64) / TRANS, 0.0, 1.0)
        per = 0.25 * (1.0 - np.cos(np.pi * tu)) * (1.0 - np.cos(np.pi * td))
        return per.prod(-1)

    num = np.zeros(n_pts, np.float64)
    den = np.zeros(n_pts, np.float64)
    bo = np.asarray(bo, np.float64)
    Wo64 = np.asarray(Wo, np.float64)
    for core in range(N_CORES):
        for p in range(PAIRS):
            oo = res.results[core][f"oo{p}"]                  # (2, pad)
            pad = slot_pads[p]
            h2t = res.results[core]["h2t"] if p == PAIRS - 1 else None
            for j, s in enumerate(assign[core][p]):
                ii = idx[s]
                if len(ii) == 0:
                    continue
                o_flat = oo[j].astype(np.float64)
                if h2t is not None:
                    o_flat[pad - 256:pad] = (
                        Wo64[s, 0] @ h2t[64 * j:64 * j + 64].astype(np.float64))
                w = window_vals(x64[ii], s)                   # (P_s,)
                num[ii] += w * (o_flat[:len(ii)] + bo[s, 0])
                den[ii] += w
    y = num / (den + TOL)
    return y.astype(np.float32).reshape(n_pts, 1)



# revision 53
# speedup vs baseline: 1.2388x; 1.2388x over previous
"""FBPINN (windowed mixture of per-subdomain MLPs) Trainium2 kernel.

Strategy: the cosine partition-of-unity window has compact support — a
point contributes to a subdomain's MLP only if it lies inside that
subdomain's box, and contributions with negligible normalized window
weight (< EPS) are dropped.  Dense evaluation would waste >90% of the
FLOPs.

  host:   route points -> per-expert gathered (padded) point lists
  device: 8 cores x 4 experts each; experts packed in pairs into the
          128-partition systolic array (block-diagonal weights)
  host:   scatter-add  w*(o+bo)  and  w  per point, final normalize.

The tanh chain on the ACT engine is the bottleneck (0.83ns/column), so
the schedule is built around keeping ACT streaming and pushing
everything else off it:
  - all matmul data is bf16.  Layer-0 stays accurate because x and the
    folded W0 are hi/lo bf16-split on the host (the 4 cross terms
    reconstruct an ~fp24 product); K grows to 18 rows (the last two are
    a ones-row pair carrying the hi/lo-split folded bias) which is free
    since PE cost scales only with moving columns.
  - the first DMA carries [W0-block | first 1024 xt columns] packed in
    one tensor so the first matmul+tanh starts as early as possible
    (PE ramps with absolute time in the p-state model; no warm-up).
  - a slice of every wide tanh chunk is evaluated on the otherwise-idle
    DVE with a clamped rational approximation (max err ~4e-3, fp16
    intermediates): clamp -> u=x^2 -> n=(u+a)x -> d=cu+b -> n/d.
  - the output layer runs transposed: 128-column chunks of h2 are the
    *stationary* operand and wo [128,2] the moving one, so Lout costs
    ~2 PE cycles per 128 points and each slot's outputs leave PSUM in
    one small DVE copy + one DMA.
  - the last 256 columns skip the device output layer: raw tanh h2
    leaves in the same final DMA and the 1x64 output matmul runs on the
    host, shortening the closing DMA chain.
"""

import numpy as np

import concourse.bacc as bacc
import concourse.mybir as mybir
import concourse.tile as tile
from concourse.bass_utils import run_bass_kernel_spmd

# problem constants (hardcoded per contract)
N_PTS = 32768
S = 32
XDIM = 2
WIDTH = 64
TRANS = 0.1
TOL = 1e-8
N_CORES = 8
E_PER_CORE = S // N_CORES
PAIRS = E_PER_CORE // 2        # 2 block-diag pairs (slots) per core

MM_CH = 512                    # moving-operand columns per matmul
ACT_CH = 1024                  # tanh chunk: 2 psum banks, triple-buffered
KST = 18                       # L0 rows: 16 hi/lo cross terms + 2 bias
H0 = 1408                      # xt columns packed into the hot first DMA
TAIL = 256                     # host-finished h2 columns of the last slot
EPS = 4e-3                     # drop instances with normalized window < EPS
CAP = 2304                     # per-expert instance cap (drops smallest w)

# DVE tanh lane: columns [0:DL[p]) of slot p run on the DVE across all
# three layers (self-chained, decoupled from the ACT pipeline)
DLS = (384, 256)
# clamped rational tanh: x*(A+x^2)/(B+C*x^2), |x| clamped to CL,
# refactored division-free as x*(K1 + K2/(x^2 + K3)) for the DVE
# (TensorTensor divide is not a valid DVE ALU op on TRN2 hardware)
TCL = 3.380041
TK1, TK2, TK3 = 0.125213207672, 2.432772338764, 2.816833243679

# packed per-pair weight layout (128 partitions x WCOLS, bf16):
#   [0:128)   W1 block-diag   [128:256) W2 block-diag
#   [256:384) W0 rows 0:18    [384:386) Wo columns (moving operand)
WCOLS = 386

_compiled_cache: dict[tuple, object] = {}


def _chunks(total, start=0, width=ACT_CH):
    out = []
    a = start
    while a < total:
        out.append((a, min(width, total - a)))
        a += width
    return out


def _build_nc(slot_pads: tuple[int, ...]):
    fp32 = mybir.dt.float32
    bf16 = mybir.dt.bfloat16
    int16 = mybir.dt.int16
    alu = mybir.AluOpType
    fp16 = mybir.dt.float16
    P0, P1 = slot_pads
    DL0, DL1 = DLS
    nc = bacc.Bacc("TRN2", target_bir_lowering=False, debug=False,
                   num_devices=N_CORES)

    # hot first transfer: [18, 128+H0] = W0-block(slot0) | xt0[:, 0:H0]
    xh0_d = nc.dram_tensor("xh0", [KST, 128 + H0], bf16, kind="ExternalInput")
    xta_d = nc.dram_tensor("xta", [KST, P0 - H0], bf16, kind="ExternalInput")
    xtb_d = nc.dram_tensor("xtb", [KST, P1], bf16, kind="ExternalInput")
    wp_d = nc.dram_tensor("wpack", [PAIRS, 128, WCOLS], bf16,
                          kind="ExternalInput")
    bias_d = nc.dram_tensor("bias", [128, 4], fp32, kind="ExternalInput")

    nch0 = P0 // 128
    nch1 = (P1 - TAIL) // 128
    # single output row layout: [oo0 | oo1 | raw h2 tail (host Lout)]
    used = 2 * nch0 + 2 * nch1 + TAIL
    oall_d = nc.dram_tensor("oall", [128, used], bf16,
                            kind="ExternalOutput")

    tanh = mybir.ActivationFunctionType.Tanh

    with tile.TileContext(nc) as tc:
        with (
            tc.tile_pool(name="wpool", bufs=1) as wpool,
            tc.tile_pool(name="hpool", bufs=1) as hpool,
            tc.tile_pool(name="dvp", bufs=2) as dvp,
            tc.tile_pool(name="ps", bufs=3, space="PSUM") as ps,
            tc.tile_pool(name="dvps", bufs=1, space="PSUM") as dvps,
            tc.tile_pool(name="psop", bufs=1, space="PSUM") as psop,
        ):
            # ---- input DMAs, latency-critical first -------------------
            xh0 = hpool.tile([KST, 128 + H0], bf16, tag="xh0")
            xta = hpool.tile([KST, P0 - H0], bf16, tag="xta")
            xtb = hpool.tile([KST, P1], bf16, tag="xtb")
            bt = wpool.tile([128, 4], fp32, tag="bt")
            wp0 = wpool.tile([128, WCOLS], bf16, tag="wp0")
            wp1 = wpool.tile([128, WCOLS], bf16, tag="wp1")
            nc.sync.dma_start(xh0[:], xh0_d[:])
            nc.sync.dma_start(xta[:], xta_d[:])
            nc.sync.dma_start(xtb[:], xtb_d[:])
            nc.sync.dma_start(bt[:], bias_d[:])
            nc.gpsimd.dma_start(wp0[:], wp_d[0])
            nc.gpsimd.dma_start(wp1[:], wp_d[1])

            # preload the tanh spline table while DMAs stream
            wm = wpool.tile([1, 1], fp32, tag="wm")
            nc.vector.memset(wm[:], 0.0)
            nc.scalar.activation(wm[0:1, 0:1], wm[0:1, 0:1], tanh)

            h = {}
            for p in range(PAIRS):
                pad = slot_pads[p]
                for ln in ("h0", "h1", "h2"):
                    h[p, ln] = hpool.tile([128, pad], bf16,
                                          tag=f"{ln}_{p}", name=f"{ln}_{p}")
            oall = hpool.tile([128, used], bf16, tag="oall")

            def dve_lane(src, soff, lw, bias, dst, dl):
                """one layer of the [0:dl) lane: matmul into the dedicated
                psum bank, then clamped rational tanh entirely on DVE:
                tanh(z) ~= zc*(K1 + K2/(zc^2+K3)), zc = clamp(z, +-CL)."""
                pdv = dvps.tile([128, max(DLS)], fp32, tag="dv", name="pdv")
                nc.tensor.matmul(pdv[:, 0:dl], lw, src[:, soff:soff + dl],
                                 start=True, stop=True)
                zc = dvp.tile([128, max(DLS)], fp16, tag="zc", name="zc")
                uu = dvp.tile([128, max(DLS)], fp16, tag="uu", name="uu")
                rr = dvp.tile([128, max(DLS)], fp16, tag="rr", name="rr")
                z, u, r = (t[:, 0:dl] for t in (zc, uu, rr))
                if bias is None:
                    nc.vector.tensor_scalar(z, pdv[:, 0:dl], TCL, -TCL,
                                            alu.min, alu.max)
                else:
                    t0 = dvp.tile([128, max(DLS)], fp16, tag="t0", name="t0")
                    nc.vector.tensor_scalar(t0[:, 0:dl], pdv[:, 0:dl],
                                            bias, TCL, alu.add, alu.min)
                    nc.vector.tensor_scalar(z, t0[:, 0:dl], -TCL, None,
                                            alu.max)
                nc.vector.tensor_tensor(u, z, z, alu.mult)
                nc.vector.tensor_scalar(u, u, TK3, None, alu.add)
                with nc.allow_low_precision("rational tanh approximation"):
                    nc.vector.reciprocal(r, u)
                nc.vector.tensor_scalar(r, r, TK2, TK1, alu.mult, alu.add)
                nc.vector.tensor_tensor(dst[:, 0:dl], r, z, alu.mult)

            def act_layer(srcs, dst, lw, bias, grid):
                """ACT-lane matmul + tanh chunks; srcs: a0 -> (ap, offset)"""
                for a0, alen in grid:
                    pst = ps.tile([128, ACT_CH], fp32, tag="ps", name="pst")
                    src, soff = srcs(a0)
                    for m in range(0, alen, MM_CH):
                        ml = min(MM_CH, alen - m)
                        nc.tensor.matmul(pst[:, m:m + ml], lw,
                                         src[:, soff + m:soff + m + ml],
                                         start=True, stop=True)
                    nc.scalar.activation(
                        dst[:, a0:a0 + alen], pst[:, 0:alen], tanh,
                        bias=bias if bias is not None else 0.0)

            def lout(h2_tile, wo, pso, base, nch):
                """transposed output layer: h2 chunks stationary, wo moving"""
                for c in range(nch):
                    nc.tensor.matmul(pso[:, base + 2 * c:base + 2 * c + 2],
                                     h2_tile[:, 128 * c:128 * c + 128],
                                     wo, start=True, stop=True)

            w0s0 = xh0[0:KST, 0:128]
            w1s0, w2s0 = wp0[:, 0:128], wp0[:, 128:256]
            w0s1 = wp1[0:KST, 256:384]
            w1s1, w2s1 = wp1[:, 0:128], wp1[:, 128:256]
            x0src = lambda a0: (xh0, 128 + a0) if a0 < H0 else (xta, a0 - H0)
            s0_l0 = [(DL0, 512), (DL0 + 512, H0 - DL0 - 512)] \
                + _chunks(P0, start=H0)
            s0_l12 = [(DL0, H0 - DL0)] + _chunks(P0, start=H0)
            grid1 = _chunks(P1, start=DL1)

            # ---- slot 0 L0 (ACT lane [DL0:P0), DVE lane [0:DL0)) ------
            act_layer(x0src, h[0, "h0"], w0s0, None, s0_l0)
            dve_lane(xh0, 128, w0s0, None, h[0, "h0"], DL0)
            # ---- slot 1 L0 on DVE early, ACT follows its own stream ---
            act_layer(lambda a0: (xtb, a0), h[1, "h0"], w0s1, None, grid1)
            dve_lane(xtb, 0, w0s1, None, h[1, "h0"], DL1)
            # ---- slot 0 L1 / slot 1 L1 --------------------------------
            act_layer(lambda a0: (h[0, "h0"], a0), h[0, "h1"], w1s0,
                      bt[:, 0:1], s0_l12)
            dve_lane(h[0, "h0"], 0, w1s0, bt[:, 0:1], h[0, "h1"], DL0)
            act_layer(lambda a0: (h[1, "h0"], a0), h[1, "h1"], w1s1,
                      bt[:, 2:3], grid1)
            dve_lane(h[1, "h0"], 0, w1s1, bt[:, 2:3], h[1, "h1"], DL1)
            # ---- L2: slot 0 runs entirely on ACT (no DVE lane), so its
            # output layer is gated only by the ACT stream and the DVE
            # queue drains early; slot 1 keeps its lane ------------------
            body = P1 - TAIL
            grid1_l2 = _chunks(body, start=DL1)
            s0_l2 = _chunks(P0)
            act_layer(lambda a0: (h[0, "h1"], a0), h[0, "h2"], w2s0,
                      bt[:, 1:2], s0_l2)
            dve_lane(h[1, "h1"], 0, w2s1, bt[:, 3:4], h[1, "h2"], DL1)
            act_layer(lambda a0: (h[1, "h1"], a0), h[1, "h2"], w2s1,
                      bt[:, 3:4], grid1_l2)

            # closing: transposed output layers through one psum bank.
            # PE order matters: lout0 first (gated by the slot-0 DVE chain,
            # which ends before the last ACT chunk), then lout1; copy1 on
            # DVE, tail tanh + slot-0 copy on the freshly-idle ACT.
            pso0 = psop.tile([128, 128], fp32, tag="pso", name="pso0")
            # slot-1's lout psum borrows the DVE-lane bank (free by now),
            # so each copy waits only on its own slot's output matmuls
            pso1 = dvps.tile([128, max(DLS)], fp32, tag="dv", name="pso1")
            pst = ps.tile([128, ACT_CH], fp32, tag="ps", name="pst_tail")
            for m in range(0, TAIL, MM_CH):
                ml = min(MM_CH, TAIL - m)
                nc.tensor.matmul(pst[:, m:m + ml], w2s1,
                                 h[1, "h1"][:, body + m:body + m + ml],
                                 start=True, stop=True)
            lout(h[0, "h2"], wp0[:, 384:386], pso0, 0, nch0)
            lout(h[1, "h2"], wp1[:, 384:386], pso1, 0, nch1)
            nc.vector.tensor_copy(oall[:, 2 * nch0:2 * (nch0 + nch1)],
                                  pso1[:, 0:2 * nch1])
            nc.scalar.activation(oall[:, 2 * (nch0 + nch1):used],
                                 pst[:, 0:TAIL], tanh, bias=bt[:, 3:4])
            nc.scalar.activation(oall[:, 0:2 * nch0], pso0[:, 0:2 * nch0],
                                 mybir.ActivationFunctionType.Copy)

            nc.sync.dma_start(oall_d[:, 0:used], oall[:])
    nc.compile()
    return nc


def _get_nc(slot_pads):
    key = tuple(slot_pads)
    nc = _compiled_cache.get(key)
    if nc is None:
        nc = _build_nc(key)
        _compiled_cache[key] = nc
    return nc


def _assign_experts(counts):
    """Pair experts and assign to (core, slot) balancing point counts."""
    order = sorted(range(S), key=lambda s: (-counts[s], s))
    pairs = [(order[2 * i], order[2 * i + 1]) for i in range(S // 2)]
    pairs.sort(key=lambda ab: -max(counts[ab[0]], counts[ab[1]]))
    assign = [[None] * PAIRS for _ in range(N_CORES)]
    slot_pads = []
    for p in range(PAIRS):
        chunk = pairs[p * N_CORES:(p + 1) * N_CORES]
        mx = max(max(counts[a], counts[b]) for a, b in chunk)
        pad = int(max(H0 + 128 if p == 0 else DLS[1] + 128,
                      -(-int(mx) // 128) * 128))
        slot_pads.append(pad)
        for c in range(N_CORES):
            assign[c][p] = chunk[c]
    return assign, tuple(slot_pads)


def _window_all(x64, xmins64, xmaxs64):
    xe = x64[:, None, :]
    tu = np.clip((xe - xmins64) / TRANS, 0.0, 1.0)
    td = np.clip((xmaxs64 - xe) / TRANS, 0.0, 1.0)
    per = 0.25 * (1.0 - np.cos(np.pi * tu)) * (1.0 - np.cos(np.pi * td))
    return per.prod(-1)                                   # (N, S)


def _kernel_numpy(x, xmins, xmaxs, W0, b0, W1, b1, W2, b2, Wo, bo):
    """Dense reference fallback (correct for any shapes, host-only)."""
    x = np.asarray(x, np.float64)
    xmins = np.asarray(xmins, np.float64)
    xmaxs = np.asarray(xmaxs, np.float64)
    w = _window_all(x, xmins, xmaxs)
    w = w / (w.sum(1, keepdims=True) + TOL)
    center = 0.5 * (xmins + xmaxs)
    scale = np.maximum(0.5 * (xmaxs - xmins), 1e-9)
    xn = (x[:, None, :] - center) / scale
    h = np.tanh(np.einsum("nsd,shd->nsh", xn, np.asarray(W0, np.float64))
                + np.asarray(b0, np.float64))
    h = np.tanh(np.einsum("nsh,skh->nsk", h, np.asarray(W1, np.float64))
                + np.asarray(b1, np.float64))
    h = np.tanh(np.einsum("nsh,skh->nsk", h, np.asarray(W2, np.float64))
                + np.asarray(b2, np.float64))
    out = (np.einsum("nsh,soh->nso", h, np.asarray(Wo, np.float64))
           + np.asarray(bo, np.float64))
    y = (out * w[:, :, None]).sum(1)
    return y.astype(np.float32)


def kernel(x, xmins, xmaxs, W0, b0, W1, b1, W2, b2, Wo, bo):
    import ml_dtypes

    bf = ml_dtypes.bfloat16
    x = np.asarray(x)
    n_pts = x.shape[0]
    args = (x, xmins, xmaxs, W0, b0, W1, b1, W2, b2, Wo, bo)
    if (x.shape != (N_PTS, XDIM) or np.asarray(xmins).shape != (S, XDIM)
            or np.asarray(W0).shape != (S, WIDTH, XDIM)):
        return _kernel_numpy(*args)

    xmins64 = np.asarray(xmins, np.float64)
    xmaxs64 = np.asarray(xmaxs, np.float64)
    x64 = np.asarray(x, np.float64)

    # ---- host routing: keep instances with non-negligible window ------
    wraw = _window_all(x64, xmins64, xmaxs64)              # (N, S)
    wsum = wraw.sum(1) + TOL
    keep = wraw > EPS * wsum[:, None]
    idx = [np.nonzero(keep[:, s])[0] for s in range(S)]
    for s in range(S):
        if len(idx[s]) > CAP:
            ii = idx[s]
            rel = wraw[ii, s] / wsum[ii]
            top = np.argpartition(rel, len(ii) - CAP)[len(ii) - CAP:]
            idx[s] = np.sort(ii[top])
    counts = np.array([len(i) for i in idx])
    assign, slot_pads = _assign_experts(counts)
    if max(slot_pads) > 6144 or slot_pads[0] <= H0:
        return _kernel_numpy(*args)

    # ---- fold input normalization into layer-0 weights (float64) ----
    center = 0.5 * (xmins64 + xmaxs64)
    scale = np.maximum(0.5 * (xmaxs64 - xmins64), 1e-9)
    W0f = np.asarray(W0, np.float64) / scale[:, None, :]
    b0f = np.asarray(b0, np.float64) - (W0f * center[:, None, :]).sum(-1)

    def _split(v):
        hi = np.asarray(v, np.float32).astype(bf).astype(np.float32)
        return hi, (np.asarray(v, np.float32) - hi)

    W1 = np.asarray(W1)
    W2 = np.asarray(W2)
    Wo = np.asarray(Wo)
    b1a = np.asarray(b1)
    b2a = np.asarray(b2)
    in_maps = []
    for core in range(N_CORES):
        m = {"wpack": np.zeros((PAIRS, 128, WCOLS), bf),
             "bias": np.zeros((128, 4), np.float32)}
        wpk = m["wpack"]
        bk = m["bias"]
        xts = []
        for p in range(PAIRS):
            pad = slot_pads[p]
            xt = np.zeros((KST, pad), bf)
            for j, s in enumerate(assign[core][p]):
                lo, hi = 64 * j, 64 * (j + 1)
                pts = x[idx[s]]                               # (P_s, 2)
                x_hi, x_lo = _split(pts.T)                    # (2, P_s)
                w_hi, w_lo = _split(W0f[s].T)                 # (2, 64)
                b_hi, b_lo = _split(b0f[s])                   # (64,)
                r0 = 8 * j
                n = pts.shape[0]
                # rows: [x_hi|W_hi, x_lo|W_hi, x_hi|W_lo, x_lo|W_lo]
                xt[r0 + 0:r0 + 2, :n] = x_hi.astype(bf)
                xt[r0 + 2:r0 + 4, :n] = x_lo.astype(bf)
                xt[r0 + 4:r0 + 6, :n] = x_hi.astype(bf)
                xt[r0 + 6:r0 + 8, :n] = x_lo.astype(bf)
                xt[16:18, :n] = 1.0
                wpk[p, lo:hi, 0 + lo:0 + hi] = W1[s].T.astype(bf)
                wpk[p, lo:hi, 128 + lo:128 + hi] = W2[s].T.astype(bf)
                wpk[p, r0 + 0:r0 + 2, 256 + lo:256 + hi] = w_hi.astype(bf)
                wpk[p, r0 + 2:r0 + 4, 256 + lo:256 + hi] = w_hi.astype(bf)
                wpk[p, r0 + 4:r0 + 6, 256 + lo:256 + hi] = w_lo.astype(bf)
                wpk[p, r0 + 6:r0 + 8, 256 + lo:256 + hi] = w_lo.astype(bf)
                wpk[p, 16, 256 + lo:256 + hi] = b_hi.astype(bf)
                wpk[p, 17, 256 + lo:256 + hi] = b_lo.astype(bf)
                wpk[p, lo:hi, 384 + j] = Wo[s, 0, :].astype(bf)
                bk[lo:hi, 2 * p + 0] = b1a[s]
                bk[lo:hi, 2 * p + 1] = b2a[s]
            xts.append(xt)
        # hot first transfer: [W0-block | first H0 cols of xt slot0]
        xh0 = np.zeros((KST, 128 + H0), bf)
        xh0[:, 0:128] = wpk[0, 0:KST, 256:384]
        xh0[:, 128:128 + H0] = xts[0][:, 0:H0]
        m["xh0"] = xh0
        m["xta"] = np.ascontiguousarray(xts[0][:, H0:])
        m["xtb"] = xts[1]
        in_maps.append(m)

    # ---- run on 8 cores ----
    global _last_in_maps
    _last_in_maps = in_maps
    try:
        nc = _get_nc(slot_pads)
        res = run_bass_kernel_spmd(nc, in_maps,
                                   core_ids=list(range(N_CORES)),
                                   trace=False)
    except Exception:
        import os
        if os.environ.get("BASS_KERNEL_NO_FALLBACK"):
            raise
        return _kernel_numpy(*args)

    # ---- host: unpack transposed Lout, finish tail, scatter-add ------
    num = np.zeros(n_pts, np.float64)
    den = np.zeros(n_pts, np.float64)
    bo = np.asarray(bo, np.float64)
    Wo64 = np.asarray(Wo, np.float64)
    P0, P1 = slot_pads
    body1 = P1 - TAIL
    nch0 = P0 // 128
    nch1 = body1 // 128
    for core in range(N_CORES):
        oa = np.asarray(res.results[core]["oall"], np.float64)
        h2t = oa[:, 2 * (nch0 + nch1):]                    # (128, TAIL)
        for p in range(PAIRS):
            if p == 0:
                oo = oa[:, 0:2 * nch0]
            else:
                oo = oa[:, 2 * nch0:2 * (nch0 + nch1)]
            nch = oo.shape[1] // 2
            # o_dev[j, q] for column q = 128*c + m  ->  oo[m, 2c+j]
            o_dev = oo.reshape(128, nch, 2).transpose(2, 1, 0) \
                .reshape(2, nch * 128)
            for j, s in enumerate(assign[core][p]):
                ii = idx[s]
                ni = len(ii)
                if ni == 0:
                    continue
                o_flat = np.empty(slot_pads[p])
                o_flat[:o_dev.shape[1]] = o_dev[j]
                if p == 1:
                    o_flat[body1:P1] = Wo64[s, 0] @ h2t[64 * j:64 * j + 64]
                w = wraw[ii, s]
                num[ii] += w * (o_flat[:ni] + bo[s, 0])
                den[ii] += w
    y = num / (den + TOL)
    return y.astype(np.float32).reshape(n_pts, 1)
